# revision 15
# baseline (speedup 1.0000x reference)
"""Multi-head causal attention kernel for Trainium2 (8 NeuronCores, batch-parallel).

Problem: B=8, Tx=Tz=1024, Dx=Dz=1024, Datt=Dmid=64, H=16, Dout=1024, causal mask.
Sharding: batch dim across the 8 cores (one batch element per core) - weights
replicated, no collectives needed.

Per-core dataflow (fp32 accumulation in PSUM everywhere):
  V     = zT16.T @ Wv + bv   fp16 matmuls; bias folded in as a rank-1
                             (ones-row x bv-row) extra accumulation step;
                             65-col/head layout with a DVE-free ones column
                             (gpsimd memset) used to produce sumexp rows.
  Q,K   fp8 DoubleRow matmuls (2x fp16 throughput on hw): xT8/zT8 and
        Wq/Wk are e4m3; contraction pairs of 128-row tiles per instruction.
        Bias is applied by the mandatory PSUM->SBUF evict (DVE
        tensor_scalar_add), which writes fp16 QT/KT for the S matmuls.
  S^T   = KT x QT per (head, z-tile): fp16 (contraction is only 64, so
        DoubleRow would not help), out [128 z, x] PSUM, causally trimmed.
  A     = exp(S^T/8) on ACT (fp16), diagonal-band masked on GpSimd.
  y^T   = V_aug.T @ A^T (fp16): psum rows 0..63 = y^T, row 64 = sumexp.
  norm  sumexp rows DMA-gathered to SBUF, one batched DVE reciprocal per
        (pair, chunk), GpSimd partition_broadcast, DVE mul -> yT fp16.
  out   = yT_cat.T @ Wp + bp (fp16 matmuls, DVE bias evict, DMA to DRAM).
"""
import sys
import types

sys.path.insert(0, "/opt/trn_rl_repo")

if "antenv.axon_hooks" not in sys.modules:
    _m = types.ModuleType("antenv.axon_hooks")
    _m.get_axon_ntff_profile_hook = lambda: None
    sys.modules["antenv.axon_hooks"] = _m

import numpy as np
import ml_dtypes

import concourse.bacc as bacc
import concourse.mybir as mybir
import concourse.tile as tile
from concourse.bass_utils import run_bass_kernel_spmd

F32 = mybir.dt.float32
FP16 = mybir.dt.float16
FP8 = mybir.dt.float8e4
DR = mybir.MatmulPerfMode.DoubleRow
E4M3 = ml_dtypes.float8_e4m3

B, T, D, E, H = 8, 1024, 1024, 64, 16
NK = D // 128          # 8 contraction tiles
NP = H // 2            # 8 head pairs
NJ = T // 128          # 8 z tiles
NC = T // 512          # 2 x chunks
SCALE = 0.125          # 1/sqrt(64)


def build_program():
    nc = bacc.Bacc("TRN2", target_bir_lowering=False, debug=False)

    xT8_d = nc.dram_tensor("xT8", [D, T], FP8, kind="ExternalInput")
    zT8_d = nc.dram_tensor("zT8", [D, T], FP8, kind="ExternalInput")
    zT16_d = nc.dram_tensor("zT16", [D, T], FP16, kind="ExternalInput")
    wq_d = nc.dram_tensor("wq8", [D, H * E], FP8, kind="ExternalInput")
    wk_d = nc.dram_tensor("wk8", [D, H * E], FP8, kind="ExternalInput")
    wv_d = nc.dram_tensor("wv", [D, H * E], FP16, kind="ExternalInput")
    wp_d = nc.dram_tensor("wp", [H * E, D], FP16, kind="ExternalInput")
    bqk_d = nc.dram_tensor("bqk", [128, 16], F32, kind="ExternalInput")
    bvb_d = nc.dram_tensor("bvb", [128, H * E], FP16, kind="ExternalInput")
    bpb_d = nc.dram_tensor("bpb", [128, D], F32, kind="ExternalInput")
    maskt_d = nc.dram_tensor("maskt", [128, 256], FP16, kind="ExternalInput")
    out_d = nc.dram_tensor("out", [T, D], F32, kind="ExternalOutput")

    Exp = mybir.ActivationFunctionType.Exp

    with tile.TileContext(nc) as tc:
        with (
            tc.tile_pool(name="big", bufs=1) as big,
            tc.tile_pool(name="qk", bufs=4) as qk,
            tc.tile_pool(name="apool", bufs=4) as apool,
            tc.tile_pool(name="norm", bufs=4) as norm,
            tc.tile_pool(name="opool", bufs=3) as opool,
            tc.tile_pool(name="cst", bufs=1) as cst,
            tc.tile_pool(name="mps", bufs=2, space="PSUM") as mps,
            tc.tile_pool(name="sps", bufs=2, space="PSUM") as sps,
            tc.tile_pool(name="yps", bufs=2, space="PSUM") as yps,
        ):
            # ---- constants ----
            bqk_t = cst.tile([128, 16], F32)
            bvb_t = cst.tile([128, H * E], FP16)
            bpb_t = cst.tile([128, D], F32)
            maskt_t = cst.tile([128, 256], FP16)
            nc.sync.dma_start(bqk_t[:], bqk_d.ap())
            nc.sync.dma_start(bvb_t[:], bvb_d.ap())
            nc.sync.dma_start(bpb_t[:], bpb_d.ap())
            nc.sync.dma_start(maskt_t[:], maskt_d.ap())

            # ---- resident activations / weights ----
            zT16_t = [big.tile([128, T], FP16, tag="z16k", bufs=NK, name=f"z16_{k}")
                      for k in range(NK)]
            xT8_t = big.tile([128, NK, T], FP8, tag="x8")
            zT8_t = big.tile([128, NK, T], FP8, tag="z8")
            wq8_t = big.tile([128, NK, H * E], FP8, tag="wq8")
            wk8_t = big.tile([128, NK, H * E], FP8, tag="wk8")
            wv_t = big.tile([128, NK, H * E], FP16, tag="wv")
            wp_t = big.tile([128, NK, D], FP16, tag="wp")
            V_t = big.tile([128, NJ, H * 65], FP16, tag="V")
            yT_t = big.tile([128, NP, T], FP16, tag="yT")

            zT16_r = zT16_d.ap().rearrange("(k p) t -> p k t", p=128)
            for k in range(NK):
                nc.sync.dma_start(zT16_t[k][:], zT16_r[:, k, :])
            nc.sync.dma_start(wv_t[:], wv_d.ap().rearrange("(k p) he -> p k he", p=128))
            nc.sync.dma_start(zT8_t[:], zT8_d.ap().rearrange("(k p) t -> p k t", p=128))
            nc.sync.dma_start(xT8_t[:], xT8_d.ap().rearrange("(k p) t -> p k t", p=128))
            nc.sync.dma_start(wq8_t[:], wq_d.ap().rearrange("(k p) he -> p k he", p=128))
            nc.sync.dma_start(wk8_t[:], wk_d.ap().rearrange("(k p) he -> p k he", p=128))
            nc.sync.dma_start(wp_t[:], wp_d.ap().rearrange("(k p) d -> p k d", p=128))

            # ones column per (j, head) for sumexp rows (V_aug col 64)
            onesf_t = cst.tile([128, 16], FP16)
            nc.gpsimd.memset(onesf_t[:], 1.0)
            for zb in range(NJ):
                ones_dst = V_t[:, zb, :].rearrange("p (h c) -> p h c", c=65)[:, :, 64:65]
                nc.vector.tensor_copy(ones_dst, onesf_t[:].rearrange("p (h c) -> p h c", c=1))

            # ---- V phase: V[z, he] = zT16.T @ Wv + bv (65-col/head layout) ----
            for vc in range(2):
                for zb in range(NJ):
                    ps = mps.tile([128, 512], F32, tag="mps")
                    for k in range(NK):
                        nc.tensor.matmul(
                            ps[:], zT16_t[k][:, zb * 128:(zb + 1) * 128],
                            wv_t[:, k, vc * 512:(vc + 1) * 512],
                            start=(k == 0), stop=(k == NK - 1),
                        )
                    dst = V_t[:, zb, vc * 520:(vc + 1) * 520].rearrange(
                        "p (h c) -> p h c", c=65)[:, :, 0:64]
                    nc.vector.tensor_add(
                        dst, ps[:].rearrange("p (h c) -> p h c", c=64),
                        bvb_t[:, vc * 512:(vc + 1) * 512].rearrange(
                            "p (h c) -> p h c", c=64))

            # ---- head-pair loop ----
            for P in range(NP):
                # Q/K projections: fp8 DoubleRow, contraction pairs of 128
                QT = qk.tile([128, T], FP16, tag="qk", name=f"QT_{P}")
                KT = qk.tile([128, T], FP16, tag="qk", name=f"KT_{P}")
                for mat, w8, a8, dst, bcol in (
                    ("q", wq8_t, xT8_t, QT, P),
                    ("k", wk8_t, zT8_t, KT, 8 + P),
                ):
                    for c in range(NC):
                        ps = mps.tile([128, 512], F32, tag="mps")
                        for half in range(2):
                            x0 = c * 512 + half * 256
                            for kp in range(NK // 2):
                                nc.tensor.matmul(
                                    ps[:, half * 256:(half + 1) * 256],
                                    w8[:, 2 * kp:2 * kp + 2, P * 128:(P + 1) * 128],
                                    a8[:, 2 * kp:2 * kp + 2, x0:x0 + 256],
                                    start=(kp == 0), stop=(kp == NK // 2 - 1),
                                    perf_mode=DR, skip_group_check=True,
                                )
                        nc.vector.tensor_scalar_add(
                            dst[:, c * 512:(c + 1) * 512], ps[:],
                            bqk_t[:, bcol:bcol + 1])

                # attention for the two heads of this pair
                for c in range(NC):
                    jlive = [j for j in range(NJ) if 128 * j <= 512 * c + 511]
                    yp = [yps.tile([65, 512], F32, tag="yps", name=f"yp{P}_{c}_{h01}")
                          for h01 in range(2)]
                    for j in jlive:
                        kband = j - 4 * c
                        x0 = 128 * max(kband, 0)
                        sp = sps.tile([128, 1024], F32, tag="sps")
                        at = apool.tile([128, 1024], FP16, tag="at")
                        for h01 in range(2):
                            hoff = 64 * h01
                            nc.tensor.matmul(
                                sp[:, h01 * 512 + x0:(h01 + 1) * 512],
                                KT[hoff:hoff + 64, j * 128:(j + 1) * 128],
                                QT[hoff:hoff + 64, c * 512 + x0:(c + 1) * 512],
                                start=True, stop=True,
                            )
                        # one exp over both heads' regions (strided 2-bank AP)
                        sp_v = sp[:].rearrange("p (h x) -> p h x", x=512)[:, :, x0:512]
                        at_v = at[:].rearrange("p (h x) -> p h x", x=512)[:, :, x0:512]
                        nc.scalar.activation(at_v, sp_v, Exp, bias=0.0, scale=SCALE)
                        if kband >= 0:
                            at_m = at[:].rearrange(
                                "p (h x) -> p h x", x=512)[:, :, x0:x0 + 128]
                            mk_m = maskt_t[:].rearrange("p (h x) -> p h x", x=128)
                            nc.vector.tensor_mul(at_m, at_m, mk_m)
                        for h01 in range(2):
                            h = 2 * P + h01
                            nc.tensor.matmul(
                                yp[h01][:, x0:512],
                                V_t[:, j, h * 65:(h + 1) * 65],
                                at[:, h01 * 512 + x0:(h01 + 1) * 512],
                                start=(j == jlive[0]), stop=(j == jlive[-1]),
                                skip_group_check=True,
                            )
                    # normalization + eviction to packed pair layout
                    for h01 in range(2):
                        hoff = 64 * h01
                        se_t = norm.tile([1, 512], F32, tag="se")
                        nc.vector.tensor_copy(se_t[:], yp[h01][64:65, :])
                        r_t = norm.tile([1, 512], F32, tag="rt")
                        nc.vector.reciprocal_approx_fast(r_t[:], se_t[:])
                        bc_t = norm.tile([64, 512], F32, tag="bc")
                        nc.gpsimd.partition_broadcast(bc_t[:], r_t[:])
                        nc.vector.tensor_mul(
                            yT_t[hoff:hoff + 64, P, c * 512:(c + 1) * 512],
                            yp[h01][0:64, :], bc_t[:])

            # ---- output projection: out = yT_cat.T @ Wp + bp ----
            for dc in range(2):
                for m in range(NJ):
                    ps = mps.tile([128, 512], F32, tag="mps")
                    for ht in range(NP):
                        nc.tensor.matmul(
                            ps[:], yT_t[:, ht, m * 128:(m + 1) * 128],
                            wp_t[:, ht, dc * 512:(dc + 1) * 512],
                            start=(ht == 0), stop=(ht == NP - 1),
                        )
                    o_t = opool.tile([128, 512], F32, tag="ot")
                    nc.vector.tensor_add(o_t[:], ps[:], bpb_t[:, dc * 512:(dc + 1) * 512])
                    nc.sync.dma_start(
                        out_d.ap()[m * 128:(m + 1) * 128, dc * 512:(dc + 1) * 512],
                        o_t[:])

    nc.compile()
    return nc


_CACHED_NC = None


def _get_program():
    global _CACHED_NC
    if _CACHED_NC is None:
        _CACHED_NC = build_program()
    return _CACHED_NC


def _prep_shared(Wq, bq, Wk, bk, Wv, bv, Wp, bp, mask):
    assert np.array_equal(
        np.asarray(mask), np.tril(np.ones((T, T), dtype=bool))
    ), "kernel specialized for causal (tril) mask"
    wq8 = np.ascontiguousarray(
        np.asarray(Wq, np.float32).transpose(1, 0, 2).reshape(D, H * E)).astype(E4M3)
    wk8 = np.ascontiguousarray(
        np.asarray(Wk, np.float32).transpose(1, 0, 2).reshape(D, H * E)).astype(E4M3)
    wv = np.ascontiguousarray(
        np.asarray(Wv, np.float32).transpose(1, 0, 2).reshape(D, H * E).astype(np.float16))
    wp = np.ascontiguousarray(np.asarray(Wp, np.float32).astype(np.float16))
    bq_c = np.asarray(bq, np.float32).reshape(-1)
    bk_c = np.asarray(bk, np.float32).reshape(-1)
    bqk = np.concatenate(
        [bq_c.reshape(8, 128).T, bk_c.reshape(8, 128).T], axis=1
    ).astype(np.float32)
    bvb = np.ascontiguousarray(np.broadcast_to(
        np.asarray(bv, np.float32).reshape(1, -1), (128, H * E)).astype(np.float16))
    bpb = np.ascontiguousarray(np.broadcast_to(
        np.asarray(bp, np.float32).reshape(1, -1), (128, D)).astype(np.float32))
    tri = np.triu(np.ones((128, 128), np.float16))  # allow z <= x
    maskt = np.concatenate([tri, tri], axis=1)      # [128, 256] for both heads
    return {
        "wq8": wq8, "wk8": wk8, "wv": wv, "wp": wp,
        "bqk": np.ascontiguousarray(bqk),
        "bvb": bvb,
        "bpb": bpb,
        "maskt": np.ascontiguousarray(maskt),
    }


def kernel(x, z, Wq, bq, Wk, bk, Wv, bv, Wp, bp, mask, _trace=False, _trace_kwargs=None):
    x = np.asarray(x, np.float32)
    z = np.asarray(z, np.float32)
    shared = _prep_shared(Wq, bq, Wk, bk, Wv, bv, Wp, bp, mask)
    in_maps = []
    for b in range(B):
        m = dict(shared)
        xt = np.ascontiguousarray(x[b].T)
        zt = np.ascontiguousarray(z[b].T)
        m["xT8"] = xt.astype(E4M3)
        m["zT8"] = zt.astype(E4M3)
        m["zT16"] = zt.astype(np.float16)
        in_maps.append(m)
    nc = _get_program()
    res = run_bass_kernel_spmd(
        nc, in_maps, core_ids=list(range(B)),
        trace=_trace, **(_trace_kwargs or {}),
    )
    out = np.stack([r["out"] for r in res.results]).astype(np.float32)
    if _trace:
        kernel.last_results = res
    return out


# revision 21
# speedup vs baseline: 1.0172x; 1.0172x over previous
"""Multi-head causal attention kernel for Trainium2 (8 NeuronCores, batch-parallel).

Problem: B=8, Tx=Tz=1024, Dx=Dz=1024, Datt=Dmid=64, H=16, Dout=1024, causal mask.
Sharding: batch dim across the 8 cores (one batch element per core) - weights
replicated, no collectives needed.

Per-core dataflow (fp32 accumulation in PSUM everywhere):
  V     = zT16.T @ Wv + bv   fp16 matmuls; bias folded in as a rank-1
                             (ones-row x bv-row) extra accumulation step;
                             65-col/head layout with a DVE-free ones column
                             (gpsimd memset) used to produce sumexp rows.
  Q,K   fp8 DoubleRow matmuls (2x fp16 throughput on hw): xT8/zT8 and
        Wq/Wk are e4m3; contraction pairs of 128-row tiles per instruction.
        Bias is applied by the mandatory PSUM->SBUF evict (DVE
        tensor_scalar_add), which writes fp16 QT/KT for the S matmuls.
  S^T   = KT x QT per (head, z-tile): fp16 (contraction is only 64, so
        DoubleRow would not help), out [128 z, x] PSUM, causally trimmed.
  A     = exp(S^T/8) on ACT (fp16), diagonal-band masked on GpSimd.
  y^T   = V_aug.T @ A^T (fp16): psum rows 0..63 = y^T, row 64 = sumexp.
  norm  sumexp rows DMA-gathered to SBUF, one batched DVE reciprocal per
        (pair, chunk), GpSimd partition_broadcast, DVE mul -> yT fp16.
  out   = yT_cat.T @ Wp + bp (fp16 matmuls, DVE bias evict, DMA to DRAM).
"""
import sys
import types

sys.path.insert(0, "/opt/trn_rl_repo")

if "antenv.axon_hooks" not in sys.modules:
    _m = types.ModuleType("antenv.axon_hooks")
    _m.get_axon_ntff_profile_hook = lambda: None
    sys.modules["antenv.axon_hooks"] = _m

import numpy as np
import ml_dtypes

import concourse.bacc as bacc
import concourse.mybir as mybir
import concourse.tile as tile
from concourse.bass_utils import run_bass_kernel_spmd

F32 = mybir.dt.float32
FP16 = mybir.dt.float16
FP8 = mybir.dt.float8e4
DR = mybir.MatmulPerfMode.DoubleRow
E4M3 = ml_dtypes.float8_e4m3

B, T, D, E, H = 8, 1024, 1024, 64, 16
NK = D // 128          # 8 contraction tiles
NP = H // 2            # 8 head pairs
NJ = T // 128          # 8 z tiles
NC = T // 512          # 2 x chunks
SCALE = 0.125          # 1/sqrt(64)


def build_program():
    nc = bacc.Bacc("TRN2", target_bir_lowering=False, debug=False)

    xT8_d = nc.dram_tensor("xT8", [D, T], FP8, kind="ExternalInput")
    zT8_d = nc.dram_tensor("zT8", [D, T], FP8, kind="ExternalInput")
    zT16_d = nc.dram_tensor("zT16", [D, T], FP16, kind="ExternalInput")
    wq_d = nc.dram_tensor("wq8", [D, H * E], FP8, kind="ExternalInput")
    wk_d = nc.dram_tensor("wk8", [D, H * E], FP8, kind="ExternalInput")
    wv_d = nc.dram_tensor("wv", [D, H * E], FP16, kind="ExternalInput")
    wp_d = nc.dram_tensor("wp", [H * E, D], FP16, kind="ExternalInput")
    bqk_d = nc.dram_tensor("bqk", [128, 16], F32, kind="ExternalInput")
    bvb_d = nc.dram_tensor("bvb", [128, H * E], FP16, kind="ExternalInput")
    bpb_d = nc.dram_tensor("bpb", [128, D], F32, kind="ExternalInput")
    maskt_d = nc.dram_tensor("maskt", [128, 256], FP16, kind="ExternalInput")
    out_d = nc.dram_tensor("out", [T, D], F32, kind="ExternalOutput")

    Exp = mybir.ActivationFunctionType.Exp

    with tile.TileContext(nc) as tc:
        with (
            tc.tile_pool(name="big", bufs=1) as big,
            tc.tile_pool(name="qk", bufs=4) as qk,
            tc.tile_pool(name="apool", bufs=4) as apool,
            tc.tile_pool(name="norm", bufs=4) as norm,
            tc.tile_pool(name="opool", bufs=3) as opool,
            tc.tile_pool(name="cst", bufs=1) as cst,
            tc.tile_pool(name="mps", bufs=2, space="PSUM") as mps,
            tc.tile_pool(name="sps", bufs=2, space="PSUM") as sps,
            tc.tile_pool(name="yps", bufs=2, space="PSUM") as yps,
        ):
            # ---- constants ----
            bqk_t = cst.tile([128, 16], F32)
            bvb_t = cst.tile([128, H * E], FP16)
            bpb_t = cst.tile([128, D], F32)
            maskt_t = cst.tile([128, 256], FP16)
            nc.sync.dma_start(bqk_t[:], bqk_d.ap())
            nc.sync.dma_start(bvb_t[:], bvb_d.ap())
            nc.sync.dma_start(bpb_t[:], bpb_d.ap())
            nc.sync.dma_start(maskt_t[:], maskt_d.ap())

            # ---- resident activations / weights ----
            zT16_t = [big.tile([128, T], FP16, tag="z16k", bufs=NK, name=f"z16_{k}")
                      for k in range(NK)]
            wv_t = [big.tile([128, H * E], FP16, tag="wvk", bufs=NK, name=f"wv_{k}")
                    for k in range(NK)]
            xT8_t = big.tile([128, NK, T], FP8, tag="x8")
            zT8_t = big.tile([128, NK, T], FP8, tag="z8")
            wq8_t = big.tile([128, NK, H * E], FP8, tag="wq8")
            wk8_t = big.tile([128, NK, H * E], FP8, tag="wk8")
            wp_t = big.tile([128, NK, D], FP16, tag="wp")
            V_t = big.tile([128, NJ, H * 65], FP16, tag="V")
            yT_t = big.tile([128, NP, T], FP16, tag="yT")

            # fine-grained preload: first V matmul only needs z16[0] + wv[0]
            zT16_r = zT16_d.ap().rearrange("(k p) t -> p k t", p=128)
            wv_r = wv_d.ap().rearrange("(k p) he -> p k he", p=128)
            for k in range(NK):
                nc.sync.dma_start(zT16_t[k][:], zT16_r[:, k, :])
                nc.sync.dma_start(wv_t[k][:], wv_r[:, k, :])
            nc.sync.dma_start(zT8_t[:], zT8_d.ap().rearrange("(k p) t -> p k t", p=128))
            nc.sync.dma_start(xT8_t[:], xT8_d.ap().rearrange("(k p) t -> p k t", p=128))
            nc.sync.dma_start(wq8_t[:], wq_d.ap().rearrange("(k p) he -> p k he", p=128))
            nc.sync.dma_start(wk8_t[:], wk_d.ap().rearrange("(k p) he -> p k he", p=128))
            nc.sync.dma_start(wp_t[:], wp_d.ap().rearrange("(k p) d -> p k d", p=128))

            # ones column per (j, head) for sumexp rows (V_aug col 64)
            onesf_t = cst.tile([128, 16], FP16)
            nc.gpsimd.memset(onesf_t[:], 1.0)
            for zb in range(NJ):
                ones_dst = V_t[:, zb, :].rearrange("p (h c) -> p h c", c=65)[:, :, 64:65]
                nc.vector.tensor_copy(ones_dst, onesf_t[:].rearrange("p (h c) -> p h c", c=1))

            # ---- V phase: V[z, he] = zT16.T @ Wv + bv (65-col/head layout) ----
            for vc in range(2):
                for zb in range(NJ):
                    ps = mps.tile([128, 512], F32, tag="mps")
                    for k in range(NK):
                        nc.tensor.matmul(
                            ps[:], zT16_t[k][:, zb * 128:(zb + 1) * 128],
                            wv_t[k][:, vc * 512:(vc + 1) * 512],
                            start=(k == 0), stop=(k == NK - 1),
                        )
                    dst = V_t[:, zb, vc * 520:(vc + 1) * 520].rearrange(
                        "p (h c) -> p h c", c=65)[:, :, 0:64]
                    nc.vector.tensor_add(
                        dst, ps[:].rearrange("p (h c) -> p h c", c=64),
                        bvb_t[:, vc * 512:(vc + 1) * 512].rearrange(
                            "p (h c) -> p h c", c=64))

            # ---- head-pair loop (QK projection software-pipelined 1 ahead) ----
            def qk_proj(P):
                QT = qk.tile([128, T], FP16, tag="qk", name=f"QT_{P}")
                KT = qk.tile([128, T], FP16, tag="qk", name=f"KT_{P}")
                for mat, w8, a8, dst, bcol in (
                    ("q", wq8_t, xT8_t, QT, P),
                    ("k", wk8_t, zT8_t, KT, 8 + P),
                ):
                    for c in range(NC):
                        ps = mps.tile([128, 512], F32, tag="mps")
                        for half in range(2):
                            x0 = c * 512 + half * 256
                            for kp in range(NK // 2):
                                nc.tensor.matmul(
                                    ps[:, half * 256:(half + 1) * 256],
                                    w8[:, 2 * kp:2 * kp + 2, P * 128:(P + 1) * 128],
                                    a8[:, 2 * kp:2 * kp + 2, x0:x0 + 256],
                                    start=(kp == 0), stop=(kp == NK // 2 - 1),
                                    perf_mode=DR, skip_group_check=True,
                                )
                        nc.vector.tensor_scalar_add(
                            dst[:, c * 512:(c + 1) * 512], ps[:],
                            bqk_t[:, bcol:bcol + 1])
                return QT, KT

            qkt = qk_proj(0)
            for P in range(NP):
                QT, KT = qkt

                # attention for the two heads of this pair
                for c in range(NC):
                    if c == 1 and P + 1 < NP:
                        qkt = qk_proj(P + 1)
                    jlive = [j for j in range(NJ) if 128 * j <= 512 * c + 511]
                    yp = [yps.tile([65, 512], F32, tag="yps", name=f"yp{P}_{c}_{h01}")
                          for h01 in range(2)]
                    for j in jlive:
                        kband = j - 4 * c
                        x0 = 128 * max(kband, 0)
                        sp = sps.tile([128, 1024], F32, tag="sps")
                        at = apool.tile([128, 1024], FP16, tag="at")
                        for h01 in range(2):
                            hoff = 64 * h01
                            nc.tensor.matmul(
                                sp[:, h01 * 512 + x0:(h01 + 1) * 512],
                                KT[hoff:hoff + 64, j * 128:(j + 1) * 128],
                                QT[hoff:hoff + 64, c * 512 + x0:(c + 1) * 512],
                                start=True, stop=True,
                            )
                        # one exp over both heads' regions (strided 2-bank AP)
                        sp_v = sp[:].rearrange("p (h x) -> p h x", x=512)[:, :, x0:512]
                        at_v = at[:].rearrange("p (h x) -> p h x", x=512)[:, :, x0:512]
                        nc.scalar.activation(at_v, sp_v, Exp, bias=0.0, scale=SCALE)
                        if kband >= 0:
                            at_m = at[:].rearrange(
                                "p (h x) -> p h x", x=512)[:, :, x0:x0 + 128]
                            mk_m = maskt_t[:].rearrange("p (h x) -> p h x", x=128)
                            nc.vector.tensor_mul(at_m, at_m, mk_m)
                        for h01 in range(2):
                            h = 2 * P + h01
                            nc.tensor.matmul(
                                yp[h01][:, x0:512],
                                V_t[:, j, h * 65:(h + 1) * 65],
                                at[:, h01 * 512 + x0:(h01 + 1) * 512],
                                start=(j == jlive[0]), stop=(j == jlive[-1]),
                                skip_group_check=True,
                            )
                    # normalization + eviction to packed pair layout
                    for h01 in range(2):
                        hoff = 64 * h01
                        se_t = norm.tile([1, 512], F32, tag="se")
                        nc.vector.tensor_copy(se_t[:], yp[h01][64:65, :])
                        r_t = norm.tile([1, 512], F32, tag="rt")
                        nc.vector.reciprocal_approx_fast(r_t[:], se_t[:])
                        bc_t = norm.tile([64, 512], F32, tag="bc")
                        nc.gpsimd.partition_broadcast(bc_t[:], r_t[:])
                        nc.vector.tensor_mul(
                            yT_t[hoff:hoff + 64, P, c * 512:(c + 1) * 512],
                            yp[h01][0:64, :], bc_t[:])

            # ---- output projection: out = yT_cat.T @ Wp + bp ----
            for dc in range(2):
                for m in range(NJ):
                    ps = mps.tile([128, 512], F32, tag="mps")
                    for ht in range(NP):
                        nc.tensor.matmul(
                            ps[:], yT_t[:, ht, m * 128:(m + 1) * 128],
                            wp_t[:, ht, dc * 512:(dc + 1) * 512],
                            start=(ht == 0), stop=(ht == NP - 1),
                        )
                    o_t = opool.tile([128, 512], F32, tag="ot")
                    nc.vector.tensor_add(o_t[:], ps[:], bpb_t[:, dc * 512:(dc + 1) * 512])
                    nc.sync.dma_start(
                        out_d.ap()[m * 128:(m + 1) * 128, dc * 512:(dc + 1) * 512],
                        o_t[:])

    nc.compile()
    return nc


_CACHED_NC = None


def _get_program():
    global _CACHED_NC
    if _CACHED_NC is None:
        _CACHED_NC = build_program()
    return _CACHED_NC


def _prep_shared(Wq, bq, Wk, bk, Wv, bv, Wp, bp, mask):
    assert np.array_equal(
        np.asarray(mask), np.tril(np.ones((T, T), dtype=bool))
    ), "kernel specialized for causal (tril) mask"
    wq8 = np.ascontiguousarray(
        np.asarray(Wq, np.float32).transpose(1, 0, 2).reshape(D, H * E)).astype(E4M3)
    wk8 = np.ascontiguousarray(
        np.asarray(Wk, np.float32).transpose(1, 0, 2).reshape(D, H * E)).astype(E4M3)
    wv = np.ascontiguousarray(
        np.asarray(Wv, np.float32).transpose(1, 0, 2).reshape(D, H * E).astype(np.float16))
    wp = np.ascontiguousarray(np.asarray(Wp, np.float32).astype(np.float16))
    bq_c = np.asarray(bq, np.float32).reshape(-1)
    bk_c = np.asarray(bk, np.float32).reshape(-1)
    bqk = np.concatenate(
        [bq_c.reshape(8, 128).T, bk_c.reshape(8, 128).T], axis=1
    ).astype(np.float32)
    bvb = np.ascontiguousarray(np.broadcast_to(
        np.asarray(bv, np.float32).reshape(1, -1), (128, H * E)).astype(np.float16))
    bpb = np.ascontiguousarray(np.broadcast_to(
        np.asarray(bp, np.float32).reshape(1, -1), (128, D)).astype(np.float32))
    tri = np.triu(np.ones((128, 128), np.float16))  # allow z <= x
    maskt = np.concatenate([tri, tri], axis=1)      # [128, 256] for both heads
    return {
        "wq8": wq8, "wk8": wk8, "wv": wv, "wp": wp,
        "bqk": np.ascontiguousarray(bqk),
        "bvb": bvb,
        "bpb": bpb,
        "maskt": np.ascontiguousarray(maskt),
    }


def kernel(x, z, Wq, bq, Wk, bk, Wv, bv, Wp, bp, mask, _trace=False, _trace_kwargs=None):
    x = np.asarray(x, np.float32)
    z = np.asarray(z, np.float32)
    shared = _prep_shared(Wq, bq, Wk, bk, Wv, bv, Wp, bp, mask)
    in_maps = []
    for b in range(B):
        m = dict(shared)
        xt = np.ascontiguousarray(x[b].T)
        zt = np.ascontiguousarray(z[b].T)
        m["xT8"] = xt.astype(E4M3)
        m["zT8"] = zt.astype(E4M3)
        m["zT16"] = zt.astype(np.float16)
        in_maps.append(m)
    nc = _get_program()
    res = run_bass_kernel_spmd(
        nc, in_maps, core_ids=list(range(B)),
        trace=_trace, **(_trace_kwargs or {}),
    )
    out = np.stack([r["out"] for r in res.results]).astype(np.float32)
    if _trace:
        kernel.last_results = res
    return out


# revision 30
# speedup vs baseline: 1.1103x; 1.0915x over previous
"""Multi-head causal attention kernel for Trainium2 (8 NeuronCores, batch-parallel).

Problem: B=8, Tx=Tz=1024, Dx=Dz=1024, Datt=Dmid=64, H=16, Dout=1024, causal mask.
Sharding: batch dim across the 8 cores (one batch element per core) - weights
replicated, no collectives needed.

Per-core dataflow (fp32 accumulation in PSUM everywhere):
  V     = zT16.T @ Wv + bv   fp16 matmuls; bias folded in as a rank-1
                             (ones-row x bv-row) extra accumulation step;
                             65-col/head layout with a DVE-free ones column
                             (gpsimd memset) used to produce sumexp rows.
  Q,K   fp8 DoubleRow matmuls (2x fp16 throughput on hw): xT8/zT8 and
        Wq/Wk are e4m3; contraction pairs of 128-row tiles per instruction.
        Bias is applied by the mandatory PSUM->SBUF evict (DVE
        tensor_scalar_add), which writes fp16 QT/KT for the S matmuls.
  S^T   = KT x QT per (head, z-tile): fp16 (contraction is only 64, so
        DoubleRow would not help), out [128 z, x] PSUM, causally trimmed.
  A     = exp(S^T/8) on ACT (fp16), diagonal-band masked on GpSimd.
  y^T   = V_aug.T @ A^T (fp16): psum rows 0..63 = y^T, row 64 = sumexp.
  norm  sumexp rows DMA-gathered to SBUF, one batched DVE reciprocal per
        (pair, chunk), GpSimd partition_broadcast, DVE mul -> yT fp16.
  out   = yT_cat.T @ Wp + bp (fp16 matmuls, DVE bias evict, DMA to DRAM).
"""
import sys
import types

sys.path.insert(0, "/opt/trn_rl_repo")

if "antenv.axon_hooks" not in sys.modules:
    _m = types.ModuleType("antenv.axon_hooks")
    _m.get_axon_ntff_profile_hook = lambda: None
    sys.modules["antenv.axon_hooks"] = _m

import numpy as np
import ml_dtypes

import concourse.bacc as bacc
import concourse.mybir as mybir
import concourse.tile as tile
from concourse.bass_utils import run_bass_kernel_spmd

F32 = mybir.dt.float32
FP16 = mybir.dt.float16
FP8 = mybir.dt.float8e4
DR = mybir.MatmulPerfMode.DoubleRow
E4M3 = ml_dtypes.float8_e4m3

B, T, D, E, H = 8, 1024, 1024, 64, 16
NK = D // 128          # 8 contraction tiles
NP = H // 2            # 8 head pairs
NJ = T // 128          # 8 z tiles
NC = T // 512          # 2 x chunks
SCALE = 0.125          # 1/sqrt(64)


def build_program():
    nc = bacc.Bacc("TRN2", target_bir_lowering=False, debug=False)

    xT8_d = nc.dram_tensor("xT8", [D, T], FP8, kind="ExternalInput")
    zT8_d = nc.dram_tensor("zT8", [D, T], FP8, kind="ExternalInput")
    zT16_d = nc.dram_tensor("zT16", [D, T], FP16, kind="ExternalInput")
    wq_d = nc.dram_tensor("wq8", [D, H * E], FP8, kind="ExternalInput")
    wk_d = nc.dram_tensor("wk8", [D, H * E], FP8, kind="ExternalInput")
    wv_d = nc.dram_tensor("wv", [D, H * E], FP16, kind="ExternalInput")
    wp_d = nc.dram_tensor("wp", [H * E, D], FP16, kind="ExternalInput")
    bqk_d = nc.dram_tensor("bqk", [128, 16], F32, kind="ExternalInput")
    bvb_d = nc.dram_tensor("bvb", [128, H * E], FP16, kind="ExternalInput")
    bpb_d = nc.dram_tensor("bpb", [128, D], F32, kind="ExternalInput")
    maskt_d = nc.dram_tensor("maskt", [128, 256], FP16, kind="ExternalInput")
    id128_d = nc.dram_tensor("id128", [128, 128], FP16, kind="ExternalInput")
    out_d = nc.dram_tensor("out", [T, D], F32, kind="ExternalOutput")

    Exp = mybir.ActivationFunctionType.Exp

    with tile.TileContext(nc) as tc:
        with (
            tc.tile_pool(name="big", bufs=1) as big,
            tc.tile_pool(name="qk", bufs=4) as qk,
            tc.tile_pool(name="apool", bufs=10) as apool,
            tc.tile_pool(name="norm", bufs=4) as norm,
            tc.tile_pool(name="opool", bufs=3) as opool,
            tc.tile_pool(name="cst", bufs=1) as cst,
            tc.tile_pool(name="mps", bufs=2, space="PSUM") as mps,
            tc.tile_pool(name="sps", bufs=2, space="PSUM") as sps,
            tc.tile_pool(name="yps", bufs=2, space="PSUM") as yps,
        ):
            # ---- constants ----
            bqk_t = cst.tile([128, 16], F32)
            bvb_t = cst.tile([128, H * E], FP16)
            bpb_t = cst.tile([128, D], F32)
            maskt_t = cst.tile([128, 256], FP16)
            id128_t = cst.tile([128, 128], FP16)
            nc.sync.dma_start(bqk_t[:], bqk_d.ap())
            nc.sync.dma_start(bvb_t[:], bvb_d.ap())
            nc.sync.dma_start(bpb_t[:], bpb_d.ap())
            nc.sync.dma_start(maskt_t[:], maskt_d.ap())
            nc.sync.dma_start(id128_t[:], id128_d.ap())

            # ---- resident activations / weights ----
            zT16_t = [big.tile([128, T], FP16, tag="z16k", bufs=NK, name=f"z16_{k}")
                      for k in range(NK)]
            wv_t = [big.tile([128, H * E], FP16, tag="wvk", bufs=NK, name=f"wv_{k}")
                    for k in range(NK)]
            xT8_t = big.tile([128, NK, T], FP8, tag="x8")
            zT8_t = big.tile([128, NK, T], FP8, tag="z8")
            wq8_t = big.tile([128, NK, H * E], FP8, tag="wq8")
            wk8_t = big.tile([128, NK, H * E], FP8, tag="wk8")
            wp_t = big.tile([128, NK, D], FP16, tag="wp")
            V_t = big.tile([128, NJ, H * 65], FP16, tag="V")
            yc_t = big.tile([128, NJ, H, 64], FP16, tag="yc")
            yT_t = big.tile([128, NP, T], FP16, tag="yT")

            # fine-grained preload: first V matmul only needs z16[0] + wv[0]
            zT16_r = zT16_d.ap().rearrange("(k p) t -> p k t", p=128)
            wv_r = wv_d.ap().rearrange("(k p) he -> p k he", p=128)
            for k in range(NK):
                nc.sync.dma_start(zT16_t[k][:], zT16_r[:, k, :])
                nc.sync.dma_start(wv_t[k][:], wv_r[:, k, :])
            nc.sync.dma_start(zT8_t[:], zT8_d.ap().rearrange("(k p) t -> p k t", p=128))
            nc.sync.dma_start(xT8_t[:], xT8_d.ap().rearrange("(k p) t -> p k t", p=128))
            nc.sync.dma_start(wq8_t[:], wq_d.ap().rearrange("(k p) he -> p k he", p=128))
            nc.sync.dma_start(wk8_t[:], wk_d.ap().rearrange("(k p) he -> p k he", p=128))
            nc.sync.dma_start(wp_t[:], wp_d.ap().rearrange("(k p) d -> p k d", p=128))

            # ones column per (j, head) for sumexp rows (V_aug col 64)
            onesf_t = cst.tile([128, 16], FP16)
            nc.gpsimd.memset(onesf_t[:], 1.0)
            for zb in range(NJ):
                ones_dst = V_t[:, zb, :].rearrange("p (h c) -> p h c", c=65)[:, :, 64:65]
                nc.vector.tensor_copy(ones_dst, onesf_t[:].rearrange("p (h c) -> p h c", c=1))

            # ---- V phase: V[z, he] = zT16.T @ Wv + bv (65-col/head layout) ----
            for vc in range(2):
                for zb in range(NJ):
                    ps = mps.tile([128, 512], F32, tag="mps")
                    for k in range(NK):
                        nc.tensor.matmul(
                            ps[:], zT16_t[k][:, zb * 128:(zb + 1) * 128],
                            wv_t[k][:, vc * 512:(vc + 1) * 512],
                            start=(k == 0), stop=(k == NK - 1),
                        )
                    dst = V_t[:, zb, vc * 520:(vc + 1) * 520].rearrange(
                        "p (h c) -> p h c", c=65)[:, :, 0:64]
                    nc.vector.tensor_add(
                        dst, ps[:].rearrange("p (h c) -> p h c", c=64),
                        bvb_t[:, vc * 512:(vc + 1) * 512].rearrange(
                            "p (h c) -> p h c", c=64))

            # ---- head-pair loop (QK projection software-pipelined 1 ahead) ----
            def qk_proj(P):
                QT = qk.tile([128, T], FP16, tag="qk", name=f"QT_{P}")
                KT = qk.tile([128, T], FP16, tag="qk", name=f"KT_{P}")
                for mat, w8, a8, dst, bcol in (
                    ("q", wq8_t, xT8_t, QT, P),
                    ("k", wk8_t, zT8_t, KT, 8 + P),
                ):
                    for c in range(NC):
                        ps = mps.tile([128, 512], F32, tag="mps")
                        for half in range(2):
                            x0 = c * 512 + half * 256
                            for kp in range(NK // 2):
                                nc.tensor.matmul(
                                    ps[:, half * 256:(half + 1) * 256],
                                    w8[:, 2 * kp:2 * kp + 2, P * 128:(P + 1) * 128],
                                    a8[:, 2 * kp:2 * kp + 2, x0:x0 + 256],
                                    start=(kp == 0), stop=(kp == NK // 2 - 1),
                                    perf_mode=DR, skip_group_check=True,
                                )
                        nc.vector.tensor_scalar_add(
                            dst[:, c * 512:(c + 1) * 512], ps[:],
                            bqk_t[:, bcol:bcol + 1])
                return QT, KT

            qkt = qk_proj(0)
            for P in range(NP):
                QT, KT = qkt

                # attention for the two heads of this pair.
                # X-orientation AV: out [128 x, 65] per (head, x-block);
                # stationary = A chunk [128 z, 128 x], moving = V_aug [128, 65]
                # so the sumexp rides V's ones column AND lands per-partition,
                # enabling per-partition (tensor_scalar) normalization.
                for c in range(NC):
                    if c == 1 and P + 1 < NP:
                        qkt = qk_proj(P + 1)
                    jlive = [j for j in range(NJ) if 128 * j <= 512 * c + 511]
                    yp = [yps.tile([128, 4, 65], F32, tag="yps",
                                   name=f"yp{P}_{c}_{h01}")
                          for h01 in range(2)]
                    at_c = {}
                    for j in jlive:
                        kband = j - 4 * c
                        x0 = 128 * max(kband, 0)
                        sp = sps.tile([128, 1024], F32, tag="sps")
                        at = apool.tile([128, 1024], FP16, tag="at")
                        at_c[j] = at
                        for h01 in range(2):
                            hoff = 64 * h01
                            nc.tensor.matmul(
                                sp[:, h01 * 512 + x0:(h01 + 1) * 512],
                                KT[hoff:hoff + 64, j * 128:(j + 1) * 128],
                                QT[hoff:hoff + 64, c * 512 + x0:(c + 1) * 512],
                                start=True, stop=True,
                            )
                        # one exp over both heads' regions (strided 2-bank AP)
                        sp_v = sp[:].rearrange("p (h x) -> p h x", x=512)[:, :, x0:512]
                        at_v = at[:].rearrange("p (h x) -> p h x", x=512)[:, :, x0:512]
                        nc.scalar.activation(at_v, sp_v, Exp, bias=0.0, scale=SCALE)
                        if kband >= 0:
                            at_m = at[:].rearrange(
                                "p (h x) -> p h x", x=512)[:, :, x0:x0 + 128]
                            mk_m = maskt_t[:].rearrange("p (h x) -> p h x", x=128)
                            nc.vector.tensor_mul(at_m, at_m, mk_m)
                        if kband >= 0:
                            # x-block m == kband is complete: accumulate its
                            # yp group over all z tiles j' <= j (sequential
                            # groups per psum tile - never interleaved)
                            m = kband
                            for h01 in range(2):
                                h = 2 * P + h01
                                jin = [jj for jj in jlive if jj <= j]
                                for jj in jin:
                                    nc.tensor.matmul(
                                        yp[h01][:, m, :],
                                        at_c[jj][:, h01 * 512 + m * 128:
                                                 h01 * 512 + (m + 1) * 128],
                                        V_t[:, jj, h * 65:(h + 1) * 65],
                                        start=(jj == jin[0]), stop=(jj == jin[-1]),
                                        skip_group_check=True,
                                    )
                    # normalization: evict, per-partition recip + scalar mul
                    for h01 in range(2):
                        h = 2 * P + h01
                        ynn = norm.tile([128, 4, 65], F32, tag="ynn")
                        nc.vector.tensor_copy(ynn[:], yp[h01][:])
                        rn = norm.tile([128, 4], F32, tag="rn")
                        nc.vector.reciprocal_approx_fast(rn[:], ynn[:, :, 64:65])
                        for m in range(4):
                            nc.vector.tensor_scalar_mul(
                                yc_t[:, 4 * c + m, h, :], ynn[:, m, 0:64],
                                rn[:, m:m + 1])

            # ---- transpose yc [x, he] -> yT [he, x], then output projection ----
            for m in range(NJ):
                tp = sps.tile([128, NP, 128], FP16, tag="sps")
                for t2 in range(NP):
                    nc.tensor.matmul(
                        tp[:, t2, :], yc_t[:, m, 2 * t2:2 * t2 + 2, :],
                        id128_t[:], is_transpose=True,
                    )
                nc.vector.tensor_copy(yT_t[:, :, m * 128:(m + 1) * 128], tp[:])
            for dc in range(2):
                for m in range(NJ):
                    ps = mps.tile([128, 512], F32, tag="mps")
                    for ht in range(NP):
                        nc.tensor.matmul(
                            ps[:], yT_t[:, ht, m * 128:(m + 1) * 128],
                            wp_t[:, ht, dc * 512:(dc + 1) * 512],
                            start=(ht == 0), stop=(ht == NP - 1),
                        )
                    o_t = opool.tile([128, 512], F32, tag="ot")
                    nc.vector.tensor_add(o_t[:], ps[:], bpb_t[:, dc * 512:(dc + 1) * 512])
                    nc.sync.dma_start(
                        out_d.ap()[m * 128:(m + 1) * 128, dc * 512:(dc + 1) * 512],
                        o_t[:])

    nc.compile()
    return nc


_CACHED_NC = None


def _get_program():
    global _CACHED_NC
    if _CACHED_NC is None:
        _CACHED_NC = build_program()
    return _CACHED_NC


def _prep_shared(Wq, bq, Wk, bk, Wv, bv, Wp, bp, mask):
    assert np.array_equal(
        np.asarray(mask), np.tril(np.ones((T, T), dtype=bool))
    ), "kernel specialized for causal (tril) mask"
    wq8 = np.ascontiguousarray(
        np.asarray(Wq, np.float32).transpose(1, 0, 2).reshape(D, H * E)).astype(E4M3)
    wk8 = np.ascontiguousarray(
        np.asarray(Wk, np.float32).transpose(1, 0, 2).reshape(D, H * E)).astype(E4M3)
    wv = np.ascontiguousarray(
        np.asarray(Wv, np.float32).transpose(1, 0, 2).reshape(D, H * E).astype(np.float16))
    wp = np.ascontiguousarray(np.asarray(Wp, np.float32).astype(np.float16))
    bq_c = np.asarray(bq, np.float32).reshape(-1)
    bk_c = np.asarray(bk, np.float32).reshape(-1)
    bqk = np.concatenate(
        [bq_c.reshape(8, 128).T, bk_c.reshape(8, 128).T], axis=1
    ).astype(np.float32)
    bvb = np.ascontiguousarray(np.broadcast_to(
        np.asarray(bv, np.float32).reshape(1, -1), (128, H * E)).astype(np.float16))
    bpb = np.ascontiguousarray(np.broadcast_to(
        np.asarray(bp, np.float32).reshape(1, -1), (128, D)).astype(np.float32))
    tri = np.triu(np.ones((128, 128), np.float16))  # allow z <= x
    maskt = np.concatenate([tri, tri], axis=1)      # [128, 256] for both heads
    return {
        "wq8": wq8, "wk8": wk8, "wv": wv, "wp": wp,
        "bqk": np.ascontiguousarray(bqk),
        "bvb": bvb,
        "bpb": bpb,
        "maskt": np.ascontiguousarray(maskt),
        "id128": np.eye(128, dtype=np.float16),
    }


def kernel(x, z, Wq, bq, Wk, bk, Wv, bv, Wp, bp, mask, _trace=False, _trace_kwargs=None):
    x = np.asarray(x, np.float32)
    z = np.asarray(z, np.float32)
    shared = _prep_shared(Wq, bq, Wk, bk, Wv, bv, Wp, bp, mask)
    in_maps = []
    for b in range(B):
        m = dict(shared)
        xt = np.ascontiguousarray(x[b].T)
        zt = np.ascontiguousarray(z[b].T)
        m["xT8"] = xt.astype(E4M3)
        m["zT8"] = zt.astype(E4M3)
        m["zT16"] = zt.astype(np.float16)
        in_maps.append(m)
    nc = _get_program()
    res = run_bass_kernel_spmd(
        nc, in_maps, core_ids=list(range(B)),
        trace=_trace, **(_trace_kwargs or {}),
    )
    out = np.stack([r["out"] for r in res.results]).astype(np.float32)
    if _trace:
        kernel.last_results = res
    return out


# revision 33
# speedup vs baseline: 1.1253x; 1.0135x over previous
"""Multi-head causal attention kernel for Trainium2 (8 NeuronCores, batch-parallel).

Problem: B=8, Tx=Tz=1024, Dx=Dz=1024, Datt=Dmid=64, H=16, Dout=1024, causal mask.
Sharding: batch dim across the 8 cores (one batch element per core) - weights
replicated, no collectives needed.

Per-core dataflow (fp32 accumulation in PSUM everywhere):
  V     = zT16.T @ Wv + bv   fp16 matmuls; bias folded in as a rank-1
                             (ones-row x bv-row) extra accumulation step;
                             65-col/head layout with a DVE-free ones column
                             (gpsimd memset) used to produce sumexp rows.
  Q,K   fp8 DoubleRow matmuls (2x fp16 throughput on hw): xT8/zT8 and
        Wq/Wk are e4m3; contraction pairs of 128-row tiles per instruction.
        Bias is applied by the mandatory PSUM->SBUF evict (DVE
        tensor_scalar_add), which writes fp16 QT/KT for the S matmuls.
  S^T   = KT x QT per (head, z-tile): fp16 (contraction is only 64, so
        DoubleRow would not help), out [128 z, x] PSUM, causally trimmed.
  A     = exp(S^T/8) on ACT (fp16), diagonal-band masked on GpSimd.
  y^T   = V_aug.T @ A^T (fp16): psum rows 0..63 = y^T, row 64 = sumexp.
  norm  sumexp rows DMA-gathered to SBUF, one batched DVE reciprocal per
        (pair, chunk), GpSimd partition_broadcast, DVE mul -> yT fp16.
  out   = yT_cat.T @ Wp + bp (fp16 matmuls, DVE bias evict, DMA to DRAM).
"""
import sys
import types

sys.path.insert(0, "/opt/trn_rl_repo")

if "antenv.axon_hooks" not in sys.modules:
    _m = types.ModuleType("antenv.axon_hooks")
    _m.get_axon_ntff_profile_hook = lambda: None
    sys.modules["antenv.axon_hooks"] = _m

import numpy as np
import ml_dtypes

import concourse.bacc as bacc
import concourse.mybir as mybir
import concourse.tile as tile
from concourse.bass_utils import run_bass_kernel_spmd

F32 = mybir.dt.float32
FP16 = mybir.dt.float16
FP8 = mybir.dt.float8e4
DR = mybir.MatmulPerfMode.DoubleRow
E4M3 = ml_dtypes.float8_e4m3

B, T, D, E, H = 8, 1024, 1024, 64, 16
NK = D // 128          # 8 contraction tiles
NP = H // 2            # 8 head pairs
NJ = T // 128          # 8 z tiles
NC = T // 512          # 2 x chunks
SCALE = 0.125          # 1/sqrt(64)


def build_program():
    nc = bacc.Bacc("TRN2", target_bir_lowering=False, debug=False)

    xT8_d = nc.dram_tensor("xT8", [D, T], FP8, kind="ExternalInput")
    zT8_d = nc.dram_tensor("zT8", [D, T], FP8, kind="ExternalInput")
    zT16_d = nc.dram_tensor("zT16", [D, T], FP16, kind="ExternalInput")
    wq_d = nc.dram_tensor("wq8", [D, H * E], FP8, kind="ExternalInput")
    wk_d = nc.dram_tensor("wk8", [D, H * E], FP8, kind="ExternalInput")
    wv_d = nc.dram_tensor("wv", [D, H * E], FP16, kind="ExternalInput")
    wp_d = nc.dram_tensor("wp", [H * E, D], FP16, kind="ExternalInput")
    bqk_d = nc.dram_tensor("bqk", [128, 16], F32, kind="ExternalInput")
    bvb_d = nc.dram_tensor("bvb", [128, H * E], FP16, kind="ExternalInput")
    bpb_d = nc.dram_tensor("bpb", [128, D], F32, kind="ExternalInput")
    maskt_d = nc.dram_tensor("maskt", [128, 256], FP16, kind="ExternalInput")
    id128_d = nc.dram_tensor("id128", [128, 128], FP16, kind="ExternalInput")
    out_d = nc.dram_tensor("out", [T, D], F32, kind="ExternalOutput")

    Exp = mybir.ActivationFunctionType.Exp

    with tile.TileContext(nc) as tc:
        with (
            tc.tile_pool(name="big", bufs=1) as big,
            tc.tile_pool(name="qk", bufs=4) as qk,
            tc.tile_pool(name="apool", bufs=10) as apool,
            tc.tile_pool(name="norm", bufs=4) as norm,
            tc.tile_pool(name="opool", bufs=3) as opool,
            tc.tile_pool(name="cst", bufs=1) as cst,
            tc.tile_pool(name="mps", bufs=2, space="PSUM") as mps,
            tc.tile_pool(name="sps", bufs=2, space="PSUM") as sps,
            tc.tile_pool(name="yps", bufs=2, space="PSUM") as yps,
        ):
            # ---- constants ----
            bqk_t = cst.tile([128, 16], F32)
            bvb_t = cst.tile([128, H * E], FP16)
            bpb_t = cst.tile([128, D], F32)
            maskt_t = cst.tile([128, 256], FP16)
            id128_t = cst.tile([128, 128], FP16)
            nc.sync.dma_start(bqk_t[:], bqk_d.ap())
            nc.sync.dma_start(bvb_t[:], bvb_d.ap())
            nc.sync.dma_start(bpb_t[:], bpb_d.ap())
            nc.sync.dma_start(maskt_t[:], maskt_d.ap())
            nc.sync.dma_start(id128_t[:], id128_d.ap())

            # ---- resident activations / weights ----
            zT16_t = [big.tile([128, T], FP16, tag="z16k", bufs=NK, name=f"z16_{k}")
                      for k in range(NK)]
            wv_t = [big.tile([128, H * E], FP16, tag="wvk", bufs=NK, name=f"wv_{k}")
                    for k in range(NK)]
            xT8_t = big.tile([128, NK, T], FP8, tag="x8")
            zT8_t = big.tile([128, NK, T], FP8, tag="z8")
            wq8_t = big.tile([128, NK, H * E], FP8, tag="wq8")
            wk8_t = big.tile([128, NK, H * E], FP8, tag="wk8")
            wp_t = big.tile([128, NK, D], FP16, tag="wp")
            V_t = big.tile([128, NJ, H * 65], FP16, tag="V")
            yc_t = big.tile([128, NJ, H, 64], FP16, tag="yc")
            yT_t = big.tile([128, NP, T], FP16, tag="yT")

            # fine-grained preload: first V matmul only needs z16[0] + wv[0]
            zT16_r = zT16_d.ap().rearrange("(k p) t -> p k t", p=128)
            wv_r = wv_d.ap().rearrange("(k p) he -> p k he", p=128)
            for k in range(NK):
                nc.sync.dma_start(zT16_t[k][:], zT16_r[:, k, :])
                nc.sync.dma_start(wv_t[k][:], wv_r[:, k, :])
            nc.sync.dma_start(zT8_t[:], zT8_d.ap().rearrange("(k p) t -> p k t", p=128))
            nc.sync.dma_start(xT8_t[:], xT8_d.ap().rearrange("(k p) t -> p k t", p=128))
            nc.sync.dma_start(wq8_t[:], wq_d.ap().rearrange("(k p) he -> p k he", p=128))
            nc.sync.dma_start(wk8_t[:], wk_d.ap().rearrange("(k p) he -> p k he", p=128))
            nc.sync.dma_start(wp_t[:], wp_d.ap().rearrange("(k p) d -> p k d", p=128))

            # ones column per (j, head) for sumexp rows (V_aug col 64)
            onesf_t = cst.tile([128, 16], FP16)
            nc.gpsimd.memset(onesf_t[:], 1.0)
            for zb in range(NJ):
                ones_dst = V_t[:, zb, :].rearrange("p (h c) -> p h c", c=65)[:, :, 64:65]
                nc.vector.tensor_copy(ones_dst, onesf_t[:].rearrange("p (h c) -> p h c", c=1))

            # ---- V phase: V[z, he] = zT16.T @ Wv + bv (65-col/head layout).
            # vc=0 (heads 0-7) up front; vc=1 (heads 8-15, first needed at
            # P=4) is deferred into P0's attention to start the pipeline early.
            def v_chunk(vc, zb):
                ps = mps.tile([128, 512], F32, tag="mps")
                for k in range(NK):
                    nc.tensor.matmul(
                        ps[:], zT16_t[k][:, zb * 128:(zb + 1) * 128],
                        wv_t[k][:, vc * 512:(vc + 1) * 512],
                        start=(k == 0), stop=(k == NK - 1),
                    )
                dst = V_t[:, zb, vc * 520:(vc + 1) * 520].rearrange(
                    "p (h c) -> p h c", c=65)[:, :, 0:64]
                nc.vector.tensor_add(
                    dst, ps[:].rearrange("p (h c) -> p h c", c=64),
                    bvb_t[:, vc * 512:(vc + 1) * 512].rearrange(
                        "p (h c) -> p h c", c=64))

            for zb in range(NJ):
                v_chunk(0, zb)

            # ---- head-pair loop (QK projection software-pipelined 1 ahead) ----
            def qk_proj(P):
                QT = qk.tile([128, T], FP16, tag="qk", name=f"QT_{P}")
                KT = qk.tile([128, T], FP16, tag="qk", name=f"KT_{P}")
                for mat, w8, a8, dst, bcol in (
                    ("q", wq8_t, xT8_t, QT, P),
                    ("k", wk8_t, zT8_t, KT, 8 + P),
                ):
                    for c in range(NC):
                        ps = mps.tile([128, 512], F32, tag="mps")
                        for half in range(2):
                            x0 = c * 512 + half * 256
                            for kp in range(NK // 2):
                                nc.tensor.matmul(
                                    ps[:, half * 256:(half + 1) * 256],
                                    w8[:, 2 * kp:2 * kp + 2, P * 128:(P + 1) * 128],
                                    a8[:, 2 * kp:2 * kp + 2, x0:x0 + 256],
                                    start=(kp == 0), stop=(kp == NK // 2 - 1),
                                    perf_mode=DR, skip_group_check=True,
                                )
                        nc.vector.tensor_scalar_add(
                            dst[:, c * 512:(c + 1) * 512], ps[:],
                            bqk_t[:, bcol:bcol + 1])
                return QT, KT

            qkt = qk_proj(0)
            for zb in range(NJ):
                v_chunk(1, zb)
            for P in range(NP):
                QT, KT = qkt

                # attention for the two heads of this pair.
                # X-orientation AV: out [128 x, 65] per (head, x-block);
                # stationary = A chunk [128 z, 128 x], moving = V_aug [128, 65]
                # so the sumexp rides V's ones column AND lands per-partition,
                # enabling per-partition (tensor_scalar) normalization.
                for c in range(NC):
                    if c == 1 and P + 1 < NP:
                        qkt = qk_proj(P + 1)
                    jlive = [j for j in range(NJ) if 128 * j <= 512 * c + 511]
                    yp = [yps.tile([128, 4, 65], F32, tag="yps",
                                   name=f"yp{P}_{c}_{h01}")
                          for h01 in range(2)]
                    at_c = {}

                    def av_group(m):
                        # x-block m: accumulate over all z tiles j' <= 4c+m
                        # (sequential groups per psum tile - never interleaved)
                        jin = [jj for jj in jlive if jj <= 4 * c + m]
                        for h01 in range(2):
                            h = 2 * P + h01
                            for jj in jin:
                                nc.tensor.matmul(
                                    yp[h01][:, m, :],
                                    at_c[jj][:, h01 * 512 + m * 128:
                                             h01 * 512 + (m + 1) * 128],
                                    V_t[:, jj, h * 65:(h + 1) * 65],
                                    start=(jj == jin[0]), stop=(jj == jin[-1]),
                                    skip_group_check=True,
                                )

                    for j in jlive:
                        kband = j - 4 * c
                        x0 = 128 * max(kband, 0)
                        sp = sps.tile([128, 1024], F32, tag="sps")
                        at = apool.tile([128, 1024], FP16, tag="at")
                        at_c[j] = at
                        for h01 in range(2):
                            hoff = 64 * h01
                            nc.tensor.matmul(
                                sp[:, h01 * 512 + x0:(h01 + 1) * 512],
                                KT[hoff:hoff + 64, j * 128:(j + 1) * 128],
                                QT[hoff:hoff + 64, c * 512 + x0:(c + 1) * 512],
                                start=True, stop=True,
                            )
                        # one exp over both heads' regions (strided 2-bank AP)
                        sp_v = sp[:].rearrange("p (h x) -> p h x", x=512)[:, :, x0:512]
                        at_v = at[:].rearrange("p (h x) -> p h x", x=512)[:, :, x0:512]
                        nc.scalar.activation(at_v, sp_v, Exp, bias=0.0, scale=SCALE)
                        if kband >= 0:
                            at_m = at[:].rearrange(
                                "p (h x) -> p h x", x=512)[:, :, x0:x0 + 128]
                            mk_m = maskt_t[:].rearrange("p (h x) -> p h x", x=128)
                            nc.vector.tensor_mul(at_m, at_m, mk_m)
                        # emit x-block m = kband-1 one j late so its last
                        # exp/mask has a full S-step of slack
                        if kband >= 1:
                            av_group(kband - 1)
                    av_group(3)
                    # normalization: evict, per-partition recip + scalar mul
                    for h01 in range(2):
                        h = 2 * P + h01
                        ynn = norm.tile([128, 4, 65], F32, tag="ynn")
                        nc.vector.tensor_copy(ynn[:], yp[h01][:])
                        rn = norm.tile([128, 4], F32, tag="rn")
                        nc.vector.reciprocal_approx_fast(rn[:], ynn[:, :, 64:65])
                        for m in range(4):
                            nc.vector.tensor_scalar_mul(
                                yc_t[:, 4 * c + m, h, :], ynn[:, m, 0:64],
                                rn[:, m:m + 1])

            # ---- transpose yc [x, he] -> yT [he, x], then output projection ----
            for m in range(NJ):
                tp = sps.tile([128, NP, 128], FP16, tag="sps")
                for t2 in range(NP):
                    nc.tensor.matmul(
                        tp[:, t2, :], yc_t[:, m, 2 * t2:2 * t2 + 2, :],
                        id128_t[:], is_transpose=True,
                    )
                nc.vector.tensor_copy(yT_t[:, :, m * 128:(m + 1) * 128], tp[:])
            for dc in range(2):
                for m in range(NJ):
                    ps = mps.tile([128, 512], F32, tag="mps")
                    for ht in range(NP):
                        nc.tensor.matmul(
                            ps[:], yT_t[:, ht, m * 128:(m + 1) * 128],
                            wp_t[:, ht, dc * 512:(dc + 1) * 512],
                            start=(ht == 0), stop=(ht == NP - 1),
                        )
                    o_t = opool.tile([128, 512], F32, tag="ot")
                    nc.vector.tensor_add(o_t[:], ps[:], bpb_t[:, dc * 512:(dc + 1) * 512])
                    nc.sync.dma_start(
                        out_d.ap()[m * 128:(m + 1) * 128, dc * 512:(dc + 1) * 512],
                        o_t[:])

    nc.compile()
    return nc


_CACHED_NC = None


def _get_program():
    global _CACHED_NC
    if _CACHED_NC is None:
        _CACHED_NC = build_program()
    return _CACHED_NC


def _prep_shared(Wq, bq, Wk, bk, Wv, bv, Wp, bp, mask):
    assert np.array_equal(
        np.asarray(mask), np.tril(np.ones((T, T), dtype=bool))
    ), "kernel specialized for causal (tril) mask"
    wq8 = np.ascontiguousarray(
        np.asarray(Wq, np.float32).transpose(1, 0, 2).reshape(D, H * E)).astype(E4M3)
    wk8 = np.ascontiguousarray(
        np.asarray(Wk, np.float32).transpose(1, 0, 2).reshape(D, H * E)).astype(E4M3)
    wv = np.ascontiguousarray(
        np.asarray(Wv, np.float32).transpose(1, 0, 2).reshape(D, H * E).astype(np.float16))
    wp = np.ascontiguousarray(np.asarray(Wp, np.float32).astype(np.float16))
    bq_c = np.asarray(bq, np.float32).reshape(-1)
    bk_c = np.asarray(bk, np.float32).reshape(-1)
    bqk = np.concatenate(
        [bq_c.reshape(8, 128).T, bk_c.reshape(8, 128).T], axis=1
    ).astype(np.float32)
    bvb = np.ascontiguousarray(np.broadcast_to(
        np.asarray(bv, np.float32).reshape(1, -1), (128, H * E)).astype(np.float16))
    bpb = np.ascontiguousarray(np.broadcast_to(
        np.asarray(bp, np.float32).reshape(1, -1), (128, D)).astype(np.float32))
    tri = np.triu(np.ones((128, 128), np.float16))  # allow z <= x
    maskt = np.concatenate([tri, tri], axis=1)      # [128, 256] for both heads
    return {
        "wq8": wq8, "wk8": wk8, "wv": wv, "wp": wp,
        "bqk": np.ascontiguousarray(bqk),
        "bvb": bvb,
        "bpb": bpb,
        "maskt": np.ascontiguousarray(maskt),
        "id128": np.eye(128, dtype=np.float16),
    }


def kernel(x, z, Wq, bq, Wk, bk, Wv, bv, Wp, bp, mask, _trace=False, _trace_kwargs=None):
    x = np.asarray(x, np.float32)
    z = np.asarray(z, np.float32)
    shared = _prep_shared(Wq, bq, Wk, bk, Wv, bv, Wp, bp, mask)
    in_maps = []
    for b in range(B):
        m = dict(shared)
        xt = np.ascontiguousarray(x[b].T)
        zt = np.ascontiguousarray(z[b].T)
        m["xT8"] = xt.astype(E4M3)
        m["zT8"] = zt.astype(E4M3)
        m["zT16"] = zt.astype(np.float16)
        in_maps.append(m)
    nc = _get_program()
    res = run_bass_kernel_spmd(
        nc, in_maps, core_ids=list(range(B)),
        trace=_trace, **(_trace_kwargs or {}),
    )
    out = np.stack([r["out"] for r in res.results]).astype(np.float32)
    if _trace:
        kernel.last_results = res
    return out
